# revision 6
# baseline (speedup 1.0000x reference)
"""Trainium2 Bass kernel for local-window multihead attention.

Problem: B=8, L=1024, C=1024, H=16 heads, head_dim=64, window_size=128
(positions attend to |i-j| <= 64). qkv in-projection + banded softmax
attention + out-projection.

Sharding: data-parallel — one batch element per NeuronCore (8 cores).

Per-core dataflow (all matmuls bf16, fp32 PSUM accumulation):
  xT (host-transposed, bf16)  --QK-proj-->  qT/ktpad   [channel, seq] layout
                              --V-proj -->  vpad       [seq, channel] layout
  attention per (head, qtile): S = Q.K^T (banded, 256-wide window), exp on
  ScalarE, mask-mul + rowsum fused on VectorE (tensor_tensor_reduce),
  normalize, PE-transpose P -> P^T, O^T = V^T @ P^T, assemble oT [c, l]
  out-proj from oT -> y [l, c] fp32.
"""

import numpy as np
import ml_dtypes

import concourse.bass as bass
import concourse.mybir as mybir
import concourse.tile as tile
from concourse import bacc
from concourse.bass_utils import run_bass_kernel_spmd
from concourse.masks import make_identity

BF16 = mybir.dt.bfloat16
F32 = mybir.dt.float32

B, L, C, H, HD = 8, 1024, 1024, 16, 64
WIN = 128  # attention window (|i-j| <= WIN//2)
NCORES = 8
NQT = L // 128          # query tiles of 128 rows
NCT = C // 128          # channel tiles
SCALE = 1.0 / 8.0       # 1/sqrt(HD)

_CACHED = {}


def _build_nc():
    nc = bacc.Bacc(
        "TRN2", target_bir_lowering=False, debug=False, num_devices=NCORES
    )

    xT_d = nc.dram_tensor("xT", [C, L], BF16, kind="ExternalInput").ap()
    wqk_d = nc.dram_tensor("wqkT", [C, 3 * C], BF16, kind="ExternalInput").ap()
    wo_d = nc.dram_tensor("woT", [C, C], BF16, kind="ExternalInput").ap()
    bqk_d = nc.dram_tensor("bqk", [2 * C], F32, kind="ExternalInput").ap()
    bv_d = nc.dram_tensor("bv", [C], F32, kind="ExternalInput").ap()
    y_d = nc.dram_tensor("y", [L, C], F32, kind="ExternalOutput").ap()

    AF = mybir.ActivationFunctionType
    ALU = mybir.AluOpType

    with tile.TileContext(nc) as tc:
        with (
            tc.tile_pool(name="const", bufs=1) as const,
            tc.tile_pool(name="work", bufs=3) as work,
            tc.tile_pool(name="psum", bufs=2, space="PSUM") as psum,
        ):
            # ---- persistent SBUF tensors ----
            wqk_s = const.tile([128, NCT, 3 * C], BF16, tag="wqk")  # [c_p, ct, o]
            wo_s = const.tile([128, NCT, C], BF16, tag="wo")
            xT_s = const.tile([128, NCT, L], BF16, tag="xT")        # [c_p, ct, l]
            qT_s = const.tile([128, NQT, L], BF16, tag="qT")        # [q-chan, ot, l]
            ktp_s = const.tile([128, NQT, L + 128], BF16, tag="ktp")  # k padded +-64
            vp_s = const.tile([128, NQT + 1, C], BF16, tag="vp")    # v rows shifted +64
            oT_s = const.tile([128, NCT, L], BF16, tag="oT")        # [c_p, ct, l]
            bqk_s = const.tile([128, 2 * C // 128], F32, tag="bqk")
            bv_row = const.tile([1, C], F32, tag="bvrow")
            bvrep_s = const.tile([128, C], F32, tag="bvrep")
            ones1_s = const.tile([1, 128], F32, tag="ones1")
            masks_s = const.tile([128, 3, 256], BF16, tag="masks")
            id01_s = const.tile([128, 128], BF16, tag="id01")

            # ---- input DMAs ----
            nc.sync.dma_start(wqk_s[:], wqk_d.rearrange("(ct p) o -> p ct o", p=128))
            nc.sync.dma_start(wo_s[:], wo_d.rearrange("(ct p) o -> p ct o", p=128))
            nc.sync.dma_start(xT_s[:], xT_d.rearrange("(ct p) l -> p ct l", p=128))
            nc.sync.dma_start(bqk_s[:], bqk_d.rearrange("(ot p) -> p ot", p=128))
            nc.sync.dma_start(bv_row[:], bv_d.rearrange("(p c) -> p c", p=1))

            # ---- constants: identity, band masks, padded-region zeros ----
            make_identity(nc, id01_s[:])
            nc.gpsimd.memset(ones1_s[:], 1.0)

            # band mask m1: valid iff 0 <= jl - il <= 128 (window cols at
            # ktp offset qi*128 .. +256, jl = col - (qi*128), il = row)
            m0, m1, m2 = (masks_s[:, i, :] for i in range(3))
            nc.gpsimd.memset(m1, 1.0)
            nc.gpsimd.affine_select(  # keep where jl - il >= 0
                m1, m1, compare_op=ALU.is_ge, fill=0.0,
                base=0, pattern=[[1, 256]], channel_multiplier=-1,
            )
            nc.gpsimd.affine_select(  # keep where 128 - jl + il >= 0
                m1, m1, compare_op=ALU.is_ge, fill=0.0,
                base=128, pattern=[[-1, 256]], channel_multiplier=1,
            )
            # qi == 0: also need jl >= 64 (left zero-pad region invalid)
            nc.vector.tensor_copy(m0, m1)
            nc.gpsimd.affine_select(
                m0, m0, compare_op=ALU.is_ge, fill=0.0,
                base=-64, pattern=[[1, 256]], channel_multiplier=0,
            )
            # qi == NQT-1: also need jl <= 191 (right zero-pad invalid)
            nc.vector.tensor_copy(m2, m1)
            nc.gpsimd.affine_select(
                m2, m2, compare_op=ALU.is_ge, fill=0.0,
                base=191, pattern=[[-1, 256]], channel_multiplier=0,
            )

            # zero the +-64 padded edges of ktpad / vpad
            for ot in range(NQT):
                nc.gpsimd.memset(ktp_s[:, ot, 0:64], 0.0)
                nc.gpsimd.memset(ktp_s[:, ot, L + 64 : L + 128], 0.0)
            nc.gpsimd.memset(vp_s[0:64, 0, :], 0.0)
            nc.gpsimd.memset(vp_s[64:128, NQT, :], 0.0)

            # replicate v-bias across partitions: ones[128,1] @ bv[1,512]
            for nt in range(2):
                ps = psum.tile([128, 512], F32, tag="proj")
                nc.tensor.matmul(
                    ps[:], lhsT=ones1_s[:], rhs=bv_row[:, nt * 512 : (nt + 1) * 512],
                    start=True, stop=True,
                )
                nc.scalar.copy(bvrep_s[:, nt * 512 : (nt + 1) * 512], ps[:])

            # ---- phase 1: Q/K projection -> qT_s / ktp_s (transposed) ----
            for ot in range(2 * NQT):
                for lt in range(2):
                    ps = psum.tile([128, 512], F32, tag="proj")
                    for ct in range(NCT):
                        nc.tensor.matmul(
                            ps[:],
                            lhsT=wqk_s[:, ct, ot * 128 : (ot + 1) * 128],
                            rhs=xT_s[:, ct, lt * 512 : (lt + 1) * 512],
                            start=(ct == 0), stop=(ct == NCT - 1),
                        )
                    if ot < NQT:
                        dest = qT_s[:, ot, lt * 512 : (lt + 1) * 512]
                    else:
                        dest = ktp_s[:, ot - NQT, 64 + lt * 512 : 64 + (lt + 1) * 512]
                    nc.scalar.activation(
                        dest, ps[:], AF.Identity, bias=bqk_s[:, ot : ot + 1]
                    )

            # ---- phase 2: V projection -> vpad (seq-major, shifted +64) ----
            for lt in range(NQT):
                for nt in range(2):
                    ps = psum.tile([128, 512], F32, tag="proj")
                    for ct in range(NCT):
                        nc.tensor.matmul(
                            ps[:],
                            lhsT=xT_s[:, ct, lt * 128 : (lt + 1) * 128],
                            rhs=wqk_s[:, ct, 2 * C + nt * 512 : 2 * C + (nt + 1) * 512],
                            start=(ct == 0), stop=(ct == NCT - 1),
                        )
                    vtmp = work.tile([128, 512], BF16, tag="vtmp")
                    nc.vector.scalar_tensor_tensor(
                        out=vtmp[:], in0=ps[:], scalar=1.0,
                        in1=bvrep_s[:, nt * 512 : (nt + 1) * 512],
                        op0=ALU.mult, op1=ALU.add,
                    )
                    sl = slice(nt * 512, (nt + 1) * 512)
                    # rows lt*128+p shift to vpad row +64: split partition halves
                    nc.sync.dma_start(vp_s[64:128, lt, sl], vtmp[0:64, :])
                    nc.sync.dma_start(vp_s[0:64, lt + 1, sl], vtmp[64:128, :])

            # ---- phase 3: banded attention ----
            for hp in range(H // 2):          # head pairs share a 128-chan tile
                for qi in range(NQT):
                    mi = 0 if qi == 0 else (2 if qi == NQT - 1 else 1)
                    # S matmuls for both heads issued back-to-back: K=64 at
                    # partition bases 0/64 -> distinct PE row groups, run
                    # concurrently in the array.
                    s_ps = [None, None]
                    for hh in range(2):
                        hb = hh * 64
                        s_ps[hh] = psum.tile([128, 256], F32, tag="s", bufs=4,
                                             name=f"s_ps{hh}")
                        nc.tensor.matmul(
                            s_ps[hh][:],
                            lhsT=qT_s[hb : hb + 64, hp, qi * 128 : (qi + 1) * 128],
                            rhs=ktp_s[hb : hb + 64, hp, qi * 128 : qi * 128 + 256],
                            start=True, stop=True,
                        )
                    for hh in range(2):
                        hb = hh * 64
                        h = hp * 2 + hh
                        p_sb = work.tile([128, 256], BF16, tag="p")
                        nc.scalar.activation(p_sb[:], s_ps[hh][:], AF.Exp,
                                             scale=SCALE)
                        # fused: pm = p * mask, rs = rowsum(pm)
                        pm = work.tile([128, 256], BF16, tag="pm")
                        rs = work.tile([128, 1], F32, tag="rs")
                        nc.vector.scalar_tensor_tensor(
                            out=pm[:], in0=p_sb[:], scalar=1.0,
                            in1=masks_s[:, mi, :],
                            op0=ALU.mult, op1=ALU.mult, accum_out=rs[:],
                        )
                        rc = work.tile([128, 1], F32, tag="rc")
                        nc.vector.reciprocal(rc[:], rs[:])
                        pn = work.tile([128, 256], BF16, tag="pn")
                        nc.vector.tensor_scalar_mul(pn[:], pm[:], rc[:])
                        # transpose P -> P^T via DMA (keeps PE/ACT free)
                        pt_sb = work.tile([128, 256], BF16, tag="pt_sb")
                        nc.sync.dma_start(pt_sb[:, 0:128], pn[:, 0:128],
                                          transpose=True)
                        nc.sync.dma_start(pt_sb[:, 128:256], pn[:, 128:256],
                                          transpose=True)
                        ot_ps = psum.tile([128, 128], F32, tag="ot")
                        o_out = ot_ps[hb : hb + 64, :]
                        nc.tensor.matmul(
                            o_out,
                            lhsT=vp_s[:, qi, h * 64 : (h + 1) * 64],
                            rhs=pt_sb[:, 0:128], start=True, stop=False,
                        )
                        nc.tensor.matmul(
                            o_out,
                            lhsT=vp_s[:, qi + 1, h * 64 : (h + 1) * 64],
                            rhs=pt_sb[:, 128:256], start=False, stop=True,
                        )
                        nc.scalar.copy(
                            oT_s[hb : hb + 64, hp, qi * 128 : (qi + 1) * 128], o_out
                        )

            # ---- phase 4: out projection -> y ----
            for lt in range(NQT):
                for mt in range(2):
                    ps = psum.tile([128, 512], F32, tag="proj")
                    for ct in range(NCT):
                        nc.tensor.matmul(
                            ps[:],
                            lhsT=oT_s[:, ct, lt * 128 : (lt + 1) * 128],
                            rhs=wo_s[:, ct, mt * 512 : (mt + 1) * 512],
                            start=(ct == 0), stop=(ct == NCT - 1),
                        )
                    yb = work.tile([128, 512], F32, tag="yb")
                    nc.scalar.copy(yb[:], ps[:])
                    nc.sync.dma_start(
                        y_d[lt * 128 : (lt + 1) * 128, mt * 512 : (mt + 1) * 512],
                        yb[:],
                    )

    nc.compile()
    return nc


def _get_nc():
    if "nc" not in _CACHED:
        _CACHED["nc"] = _build_nc()
    return _CACHED["nc"]


def _prep_in_maps(x, in_proj_w, in_proj_b, out_w):
    bf = ml_dtypes.bfloat16
    wqkT = np.ascontiguousarray(in_proj_w.T).astype(bf)
    woT = np.ascontiguousarray(out_w.T).astype(bf)
    bqk = np.ascontiguousarray(in_proj_b[: 2 * C]).astype(np.float32)
    bv = np.ascontiguousarray(in_proj_b[2 * C :]).astype(np.float32)
    in_maps = []
    for b in range(B):
        xT = np.ascontiguousarray(x[b].T).astype(bf)
        in_maps.append(
            {"xT": xT, "wqkT": wqkT, "woT": woT, "bqk": bqk, "bv": bv}
        )
    return in_maps


def kernel(x, in_proj_w, in_proj_b, out_w, out_b, _trace=False):
    nc = _get_nc()
    in_maps = _prep_in_maps(x, in_proj_w, in_proj_b, out_w)
    res = run_bass_kernel_spmd(nc, in_maps, list(range(NCORES)), trace=_trace)
    _CACHED["last_result"] = res
    y = np.stack([res.results[i]["y"] for i in range(NCORES)], axis=0)
    return (y + out_b[None, None, :].astype(np.float32)).astype(np.float32)


# revision 8
# speedup vs baseline: 1.6127x; 1.6127x over previous
"""Trainium2 Bass kernel for local-window multihead attention.

Problem: B=8, L=1024, C=1024, H=16 heads, head_dim=64, window_size=128
(positions attend to |i-j| <= 64). qkv in-projection + banded softmax
attention + out-projection.

Sharding: data-parallel - one batch element per NeuronCore (8 cores).

Per-core dataflow (bf16 matmuls, fp32 PSUM accumulation):
  xT (host-transposed bf16) --QK-proj--> qT/ktpad  [channel, seq] layout
                            --V-proj --> vpad      [seq, channel] layout
  attention is organized K-MAJOR: for each (head, key-chunk kc) compute
  St = K_chunk^T Q_window directly in [key, query] layout (no transposes),
  exp on ScalarE, multiplicative band mask on VectorE, then
  O^T += V_chunk^T @ P_t (PSUM-accumulated across overlapping chunks) and
  rowsums via an indicator matmul into one persistent [16, L] PSUM tile.
  Softmax normalization is deferred: one reciprocal + replicated-scale
  matmuls + fused multiply into oT at the end.
  out-proj from oT [c, l] -> y [l, c] fp32.
"""

import numpy as np
import ml_dtypes

import concourse.bass as bass
import concourse.mybir as mybir
import concourse.tile as tile
from concourse import bacc
from concourse.bass_utils import run_bass_kernel_spmd

BF16 = mybir.dt.bfloat16
F32 = mybir.dt.float32

B, L, C, H, HD = 8, 1024, 1024, 16, 64
WIN = 128  # attention window (|i-j| <= WIN//2)
NCORES = 8
NQT = L // 128          # 128-row tiles
NCT = C // 128
NKC = NQT + 1           # key chunks in padded [-64, L+64) key space
SCALE = 1.0 / 8.0       # 1/sqrt(HD)

_CACHED = {}


def _build_nc():
    nc = bacc.Bacc(
        "TRN2", target_bir_lowering=False, debug=False, num_devices=NCORES
    )

    xT_d = nc.dram_tensor("xT", [C, L], BF16, kind="ExternalInput").ap()
    wqk_d = nc.dram_tensor("wqkT", [C, 3 * C], BF16, kind="ExternalInput").ap()
    wo_d = nc.dram_tensor("woT", [C, C], BF16, kind="ExternalInput").ap()
    bqk_d = nc.dram_tensor("bqk", [2 * C], F32, kind="ExternalInput").ap()
    bv_d = nc.dram_tensor("bv", [C], F32, kind="ExternalInput").ap()
    y_d = nc.dram_tensor("y", [L, C], F32, kind="ExternalOutput").ap()

    AF = mybir.ActivationFunctionType
    ALU = mybir.AluOpType

    with tile.TileContext(nc) as tc:
        with (
            tc.tile_pool(name="const", bufs=1) as const,
            tc.tile_pool(name="work", bufs=3) as work,
        ):
            # ---- persistent SBUF tensors ----
            wqk_s = const.tile([128, NCT, 3 * C], BF16, tag="wqk")  # [c_p, ct, o]
            wo_s = const.tile([128, NCT, C], BF16, tag="wo")
            xT_s = const.tile([128, NCT, L], BF16, tag="xT")        # [c_p, ct, l]
            qT_s = const.tile([128, NQT, L], BF16, tag="qT")        # [q-chan, ot, l]
            ktp_s = const.tile([128, NQT, L + 128], BF16, tag="ktp")  # k padded
            vp_s = const.tile([128, NKC, C], BF16, tag="vp")        # v rows +64
            oT_s = const.tile([128, NCT, L], BF16, tag="oT")        # [c_p, ct, l]
            bqk_s = const.tile([128, 2 * C // 128], F32, tag="bqk")
            bv_row = const.tile([1, C], F32, tag="bvrow")
            bvrep_s = const.tile([128, C], F32, tag="bvrep")
            ones1_s = const.tile([1, 128], F32, tag="ones1")
            zrow_s = const.tile([1, 512], BF16, tag="zrow")
            masks_s = const.tile([128, 3, 256], BF16, tag="masks")
            e16c_s = const.tile([128, H, H], BF16, tag="e16c")   # rs indicator
            e16r_s = const.tile([H, NCT * 128], BF16, tag="e16r")  # rc replicate
            rc_f32 = const.tile([H, L], F32, tag="rcf")
            rc_bf = const.tile([H, L], BF16, tag="rcb")

            # ---- input DMAs ----
            nc.sync.dma_start(wqk_s[:], wqk_d.rearrange("(ct p) o -> p ct o", p=128))
            nc.sync.dma_start(wo_s[:], wo_d.rearrange("(ct p) o -> p ct o", p=128))
            nc.sync.dma_start(xT_s[:], xT_d.rearrange("(ct p) l -> p ct l", p=128))
            nc.sync.dma_start(bqk_s[:], bqk_d.rearrange("(ot p) -> p ot", p=128))
            nc.sync.dma_start(bv_row[:], bv_d.rearrange("(p c) -> p c", p=1))

            # ---- constants ----
            nc.gpsimd.memset(ones1_s[:], 1.0)
            nc.gpsimd.memset(zrow_s[:], 0.0)

            # band mask (kc interior): valid iff 0 <= jq - p <= 128
            m_band = masks_s[:, 0, :]
            nc.gpsimd.memset(m_band, 1.0)
            nc.gpsimd.affine_select(
                m_band, m_band, compare_op=ALU.is_ge, fill=0.0,
                base=0, pattern=[[1, 256]], channel_multiplier=-1,
            )
            nc.gpsimd.affine_select(
                m_band, m_band, compare_op=ALU.is_ge, fill=0.0,
                base=128, pattern=[[-1, 256]], channel_multiplier=1,
            )
            # kc = 0 (cols [0,128)): valid iff p >= jq and p >= 64
            m_lo = masks_s[:, 1, 0:128]
            nc.gpsimd.memset(m_lo, 1.0)
            nc.gpsimd.affine_select(
                m_lo, m_lo, compare_op=ALU.is_ge, fill=0.0,
                base=0, pattern=[[-1, 128]], channel_multiplier=1,
            )
            nc.gpsimd.memset(masks_s[0:64, 1, 0:128], 0.0)
            # kc = NKC-1 (cols [0,128)): valid iff jq >= p and p < 64
            m_hi = masks_s[:, 2, 0:128]
            nc.gpsimd.memset(m_hi, 1.0)
            nc.gpsimd.affine_select(
                m_hi, m_hi, compare_op=ALU.is_ge, fill=0.0,
                base=0, pattern=[[1, 128]], channel_multiplier=-1,
            )
            nc.gpsimd.memset(masks_s[64:128, 2, 0:128], 0.0)

            # e16c[p, h, j] = (j == h): lhsT picking rowsum row h
            nc.gpsimd.memset(e16c_s[:], 1.0)
            nc.gpsimd.affine_select(
                e16c_s[:], e16c_s[:], compare_op=ALU.is_equal, fill=0.0,
                base=0, pattern=[[-1, H], [1, H]], channel_multiplier=0,
            )
            # e16r[j, ct*128+m] = (j == 2*ct + m//64): replicates rc rows
            nc.gpsimd.memset(e16r_s[:], 1.0)
            nc.gpsimd.affine_select(
                e16r_s[:], e16r_s[:], compare_op=ALU.is_equal, fill=0.0,
                base=0, pattern=[[2, NCT], [1, 2], [0, 64]],
                channel_multiplier=-1,
            )

            # zero padded edges of ktpad / vpad
            for ot in range(NQT):
                nc.gpsimd.memset(ktp_s[:, ot, 0:64], 0.0)
                nc.gpsimd.memset(ktp_s[:, ot, L + 64 : L + 128], 0.0)
            nc.gpsimd.memset(vp_s[0:64, 0, :], 0.0)
            nc.gpsimd.memset(vp_s[64:128, NKC - 1, :], 0.0)

            # ---- phases 1-2: projections ----
            with tc.tile_pool(name="psA", bufs=2, space="PSUM") as psA:
                # replicate v-bias across partitions: ones[128,1] @ bv[1,512]
                for nt in range(2):
                    ps = psA.tile([128, 512], F32, tag="proj")
                    nc.tensor.matmul(
                        ps[:], lhsT=ones1_s[:],
                        rhs=bv_row[:, nt * 512 : (nt + 1) * 512],
                        start=True, stop=True,
                    )
                    nc.scalar.copy(bvrep_s[:, nt * 512 : (nt + 1) * 512], ps[:])

                # Q/K projection -> qT_s / ktp_s (transposed layouts)
                for ot in range(2 * NQT):
                    for lt in range(2):
                        ps = psA.tile([128, 512], F32, tag="proj")
                        for ct in range(NCT):
                            nc.tensor.matmul(
                                ps[:],
                                lhsT=wqk_s[:, ct, ot * 128 : (ot + 1) * 128],
                                rhs=xT_s[:, ct, lt * 512 : (lt + 1) * 512],
                                start=(ct == 0), stop=(ct == NCT - 1),
                            )
                        if ot < NQT:
                            dest = qT_s[:, ot, lt * 512 : (lt + 1) * 512]
                        else:
                            dest = ktp_s[:, ot - NQT,
                                         64 + lt * 512 : 64 + (lt + 1) * 512]
                        nc.scalar.activation(
                            dest, ps[:], AF.Identity, bias=bqk_s[:, ot : ot + 1]
                        )

                # V projection -> vpad (seq-major, shifted +64)
                for lt in range(NQT):
                    for nt in range(2):
                        ps = psA.tile([128, 512], F32, tag="proj")
                        for ct in range(NCT):
                            nc.tensor.matmul(
                                ps[:],
                                lhsT=xT_s[:, ct, lt * 128 : (lt + 1) * 128],
                                rhs=wqk_s[:, ct,
                                          2 * C + nt * 512 : 2 * C + (nt + 1) * 512],
                                start=(ct == 0), stop=(ct == NCT - 1),
                            )
                        vtmp = work.tile([128, 512], BF16, tag="vtmp")
                        nc.vector.scalar_tensor_tensor(
                            out=vtmp[:], in0=ps[:], scalar=1.0,
                            in1=bvrep_s[:, nt * 512 : (nt + 1) * 512],
                            op0=ALU.mult, op1=ALU.add,
                        )
                        sl = slice(nt * 512, (nt + 1) * 512)
                        nc.sync.dma_start(vp_s[64:128, lt, sl], vtmp[0:64, :])
                        nc.sync.dma_start(vp_s[0:64, lt + 1, sl], vtmp[64:128, :])

            # ---- phase 3: banded attention, k-major ----
            with tc.tile_pool(name="psB", bufs=1, space="PSUM") as psB:
                # rowsums for all heads in ONE psum bank: rows h (q<512)
                # and rows 32+h (q>=512)
                rs_ps = psB.tile([64, 512], F32, tag="rsall")
                nc.tensor.matmul(
                    rs_ps[:], lhsT=zrow_s[:, 0:64], rhs=zrow_s[:],
                    start=True, stop=True, skip_group_check=True,
                )

                def kc_geom(kc):
                    # query window cols for key chunk kc, in real q coords
                    q0 = max(kc * 128 - 128, 0)
                    q1 = min(kc * 128 + 128, L)
                    mi = 1 if kc == 0 else (2 if kc == NKC - 1 else 0)
                    return q0, q1, mi

                pend = []  # software pipeline: deferred AV/rs stages
                LAG = 2

                def do_av(st):
                    (h, hb, hp, kc, pm, q0, q1, ot_ps, first) = st
                    if first:  # zero-init this pair's ot banks
                        for bk in range(2):
                            nc.tensor.matmul(
                                ot_ps[:, bk * 512 : (bk + 1) * 512],
                                lhsT=zrow_s[:, 0:128], rhs=zrow_s[:],
                                start=True, stop=True, skip_group_check=True,
                            )
                    # split AV/rs matmuls at psum bank boundaries
                    spans = []
                    for b0 in (0, 512):
                        s0, s1 = max(q0, b0), min(q1, b0 + 512)
                        if s0 < s1:
                            spans.append((s0, s1))
                    for s0, s1 in spans:
                        j0, j1 = s0 - q0, s1 - q0
                        nc.tensor.matmul(
                            ot_ps[hb : hb + 64, s0:s1],
                            lhsT=vp_s[:, kc, h * 64 : (h + 1) * 64],
                            rhs=pm[:, j0:j1],
                            start=False, stop=(kc == NKC - 1),
                            skip_group_check=True,
                        )
                    for s0, s1 in spans:
                        j0, j1 = s0 - q0, s1 - q0
                        rb = 0 if s0 < 512 else 32
                        nc.tensor.matmul(
                            rs_ps[rb : rb + H, s0 - (512 if rb else 0)
                                  : s1 - (512 if rb else 0)],
                            lhsT=e16c_s[:, h, :],
                            rhs=pm[:, j0:j1],
                            start=False,
                            stop=(h == H - 1 and kc == NKC - 1),
                            skip_group_check=True,
                        )

                for hp in range(H // 2):
                    ot_ps = psB.tile([128, L], F32, tag="ot", bufs=1,
                                     name=f"ot_ps{hp}")
                    for hh in range(2):
                        hb = hh * 64
                        h = hp * 2 + hh
                        for kc in range(NKC):
                            q0, q1, mi = kc_geom(kc)
                            w = q1 - q0
                            s_ps = psB.tile([128, 256], F32, tag="s", bufs=5,
                                            name=f"s{h}_{kc}")
                            nc.tensor.matmul(
                                s_ps[:, 0:w],
                                lhsT=ktp_s[hb : hb + 64, hp,
                                           kc * 128 : (kc + 1) * 128],
                                rhs=qT_s[hb : hb + 64, hp, q0:q1],
                                start=True, stop=True,
                            )
                            p_sb = work.tile([128, 256], BF16, tag="p")
                            nc.scalar.activation(p_sb[:, 0:w], s_ps[:, 0:w],
                                                 AF.Exp, scale=SCALE)
                            pm = work.tile([128, 256], BF16, tag="pm", bufs=4)
                            nc.vector.tensor_mul(pm[:, 0:w], p_sb[:, 0:w],
                                                 masks_s[:, mi, 0:w])
                            pend.append((h, hb, hp, kc, pm, q0, q1, ot_ps,
                                         hh == 0 and kc == 0))
                            if len(pend) > LAG:
                                do_av(pend.pop(0))
                    while pend:
                        do_av(pend.pop(0))
                    # evacuate this pair's O^T (unnormalized)
                    for lt in range(2):
                        nc.vector.tensor_copy(
                            oT_s[:, hp, lt * 512 : (lt + 1) * 512],
                            ot_ps[:, lt * 512 : (lt + 1) * 512],
                        )

                # deferred softmax normalization
                nc.vector.reciprocal(rc_f32[:, 0:512], rs_ps[0:H, :])
                nc.vector.reciprocal(rc_f32[:, 512:L], rs_ps[32 : 32 + H, :])
                nc.vector.tensor_copy(rc_bf[:], rc_f32[:])

            # ---- phase 4: normalize oT + out projection -> y ----
            with tc.tile_pool(name="psC", bufs=2, space="PSUM") as psC:
                for ct in range(NCT):
                    for lt in range(2):
                        rep = psC.tile([128, 512], F32, tag="rep")
                        nc.tensor.matmul(
                            rep[:],
                            lhsT=e16r_s[:, ct * 128 : (ct + 1) * 128],
                            rhs=rc_bf[:, lt * 512 : (lt + 1) * 512],
                            start=True, stop=True,
                        )
                        nc.vector.tensor_mul(
                            oT_s[:, ct, lt * 512 : (lt + 1) * 512], rep[:],
                            oT_s[:, ct, lt * 512 : (lt + 1) * 512],
                        )
                for lt in range(NQT):
                    for mt in range(2):
                        ps = psC.tile([128, 512], F32, tag="proj")
                        for ct in range(NCT):
                            nc.tensor.matmul(
                                ps[:],
                                lhsT=oT_s[:, ct, lt * 128 : (lt + 1) * 128],
                                rhs=wo_s[:, ct, mt * 512 : (mt + 1) * 512],
                                start=(ct == 0), stop=(ct == NCT - 1),
                            )
                        yb = work.tile([128, 512], F32, tag="yb")
                        nc.scalar.copy(yb[:], ps[:])
                        nc.sync.dma_start(
                            y_d[lt * 128 : (lt + 1) * 128,
                                mt * 512 : (mt + 1) * 512],
                            yb[:],
                        )

    nc.compile()
    return nc


def _get_nc():
    if "nc" not in _CACHED:
        _CACHED["nc"] = _build_nc()
    return _CACHED["nc"]


def _prep_in_maps(x, in_proj_w, in_proj_b, out_w):
    bf = ml_dtypes.bfloat16
    wqkT = np.ascontiguousarray(in_proj_w.T).astype(bf)
    woT = np.ascontiguousarray(out_w.T).astype(bf)
    bqk = np.ascontiguousarray(in_proj_b[: 2 * C]).astype(np.float32)
    bv = np.ascontiguousarray(in_proj_b[2 * C :]).astype(np.float32)
    in_maps = []
    for b in range(B):
        xT = np.ascontiguousarray(x[b].T).astype(bf)
        in_maps.append(
            {"xT": xT, "wqkT": wqkT, "woT": woT, "bqk": bqk, "bv": bv}
        )
    return in_maps


def kernel(x, in_proj_w, in_proj_b, out_w, out_b, _trace=False):
    nc = _get_nc()
    in_maps = _prep_in_maps(x, in_proj_w, in_proj_b, out_w)
    res = run_bass_kernel_spmd(nc, in_maps, list(range(NCORES)), trace=_trace)
    _CACHED["last_result"] = res
    y = np.stack([res.results[i]["y"] for i in range(NCORES)], axis=0)
    return (y + out_b[None, None, :].astype(np.float32)).astype(np.float32)


# revision 9
# speedup vs baseline: 1.7472x; 1.0834x over previous
"""Trainium2 Bass kernel for local-window multihead attention.

Problem: B=8, L=1024, C=1024, H=16 heads, head_dim=64, window_size=128
(positions attend to |i-j| <= 64). qkv in-projection + banded softmax
attention + out-projection.

Sharding: data-parallel - one batch element per NeuronCore (8 cores).

Per-core dataflow (bf16 matmuls, fp32 PSUM accumulation):
  xT (host-transposed bf16) --QK-proj--> qT/ktpad  [channel, seq] layout
                            --V-proj --> vpad      [seq, channel] layout
  attention is organized K-MAJOR: for each (head, key-chunk kc) compute
  St = K_chunk^T Q_window directly in [key, query] layout (no transposes),
  exp on ScalarE, multiplicative band mask on VectorE, then
  O^T += V_chunk^T @ P_t (PSUM-accumulated across overlapping chunks) and
  rowsums via an indicator matmul into one persistent [16, L] PSUM tile.
  Softmax normalization is deferred: one reciprocal + replicated-scale
  matmuls + fused multiply into oT at the end.
  out-proj from oT [c, l] -> y [l, c] fp32.
"""

import numpy as np
import ml_dtypes

import concourse.bass as bass
import concourse.mybir as mybir
import concourse.tile as tile
from concourse import bacc
from concourse.bass_utils import run_bass_kernel_spmd

BF16 = mybir.dt.bfloat16
F32 = mybir.dt.float32

B, L, C, H, HD = 8, 1024, 1024, 16, 64
WIN = 128  # attention window (|i-j| <= WIN//2)
NCORES = 8
NQT = L // 128          # 128-row tiles
NCT = C // 128
NKC = NQT + 1           # key chunks in padded [-64, L+64) key space
SCALE = 1.0 / 8.0       # 1/sqrt(HD)

_CACHED = {}


def _build_nc():
    nc = bacc.Bacc(
        "TRN2", target_bir_lowering=False, debug=False, num_devices=NCORES
    )

    xT_d = nc.dram_tensor("xT", [C, L], BF16, kind="ExternalInput").ap()
    wqk_d = nc.dram_tensor("wqkT", [C, 3 * C], BF16, kind="ExternalInput").ap()
    wo_d = nc.dram_tensor("woT", [C, C], BF16, kind="ExternalInput").ap()
    bqk_d = nc.dram_tensor("bqk", [2 * C], F32, kind="ExternalInput").ap()
    bv_d = nc.dram_tensor("bv", [C], F32, kind="ExternalInput").ap()
    y_d = nc.dram_tensor("y", [L, C], F32, kind="ExternalOutput").ap()

    AF = mybir.ActivationFunctionType
    ALU = mybir.AluOpType

    with tile.TileContext(nc) as tc:
        with (
            tc.tile_pool(name="const", bufs=1) as const,
            tc.tile_pool(name="work", bufs=3) as work,
        ):
            # ---- persistent SBUF tensors ----
            wqk_s = const.tile([128, NCT, 3 * C], BF16, tag="wqk")  # [c_p, ct, o]
            wo_s = const.tile([128, NCT, C], BF16, tag="wo")
            xT_s = const.tile([128, NCT, L], BF16, tag="xT")        # [c_p, ct, l]
            qT_s = const.tile([128, NQT, L], BF16, tag="qT")        # [q-chan, ot, l]
            ktp_s = const.tile([128, NQT, L + 128], BF16, tag="ktp")  # k padded
            vp_s = const.tile([128, NKC, C], BF16, tag="vp")        # v rows +64
            oT_s = const.tile([128, NCT, L], BF16, tag="oT")        # [c_p, ct, l]
            bqk_s = const.tile([128, 2 * C // 128], F32, tag="bqk")
            bv_row = const.tile([1, C], F32, tag="bvrow")
            bvrep_s = const.tile([128, C], F32, tag="bvrep")
            ones1_s = const.tile([1, 128], F32, tag="ones1")
            zrow_s = const.tile([1, 512], BF16, tag="zrow")
            masks_s = const.tile([128, 3, 256], BF16, tag="masks")
            e16c_s = const.tile([128, H, H], BF16, tag="e16c")   # rs indicator
            e16r_s = const.tile([H, NCT * 128], BF16, tag="e16r")  # rc replicate
            rc_f32 = const.tile([H, L], F32, tag="rcf")
            rc_bf = const.tile([H, L], BF16, tag="rcb")

            # ---- input DMAs ----
            nc.sync.dma_start(wqk_s[:], wqk_d.rearrange("(ct p) o -> p ct o", p=128))
            nc.sync.dma_start(wo_s[:], wo_d.rearrange("(ct p) o -> p ct o", p=128))
            nc.sync.dma_start(xT_s[:], xT_d.rearrange("(ct p) l -> p ct l", p=128))
            nc.sync.dma_start(bqk_s[:], bqk_d.rearrange("(ot p) -> p ot", p=128))
            nc.sync.dma_start(bv_row[:], bv_d.rearrange("(p c) -> p c", p=1))

            # ---- constants ----
            nc.gpsimd.memset(ones1_s[:], 1.0)
            nc.gpsimd.memset(zrow_s[:], 0.0)

            # band mask (kc interior): valid iff 0 <= jq - p <= 128
            m_band = masks_s[:, 0, :]
            nc.gpsimd.memset(m_band, 1.0)
            nc.gpsimd.affine_select(
                m_band, m_band, compare_op=ALU.is_ge, fill=0.0,
                base=0, pattern=[[1, 256]], channel_multiplier=-1,
            )
            nc.gpsimd.affine_select(
                m_band, m_band, compare_op=ALU.is_ge, fill=0.0,
                base=128, pattern=[[-1, 256]], channel_multiplier=1,
            )
            # kc = 0 (cols [0,128)): valid iff p >= jq and p >= 64
            m_lo = masks_s[:, 1, 0:128]
            nc.gpsimd.memset(m_lo, 1.0)
            nc.gpsimd.affine_select(
                m_lo, m_lo, compare_op=ALU.is_ge, fill=0.0,
                base=0, pattern=[[-1, 128]], channel_multiplier=1,
            )
            nc.gpsimd.memset(masks_s[0:64, 1, 0:128], 0.0)
            # kc = NKC-1 (cols [0,128)): valid iff jq >= p and p < 64
            m_hi = masks_s[:, 2, 0:128]
            nc.gpsimd.memset(m_hi, 1.0)
            nc.gpsimd.affine_select(
                m_hi, m_hi, compare_op=ALU.is_ge, fill=0.0,
                base=0, pattern=[[1, 128]], channel_multiplier=-1,
            )
            nc.gpsimd.memset(masks_s[64:128, 2, 0:128], 0.0)

            # e16c[p, h, j] = (j == h): lhsT picking rowsum row h
            nc.gpsimd.memset(e16c_s[:], 1.0)
            nc.gpsimd.affine_select(
                e16c_s[:], e16c_s[:], compare_op=ALU.is_equal, fill=0.0,
                base=0, pattern=[[-1, H], [1, H]], channel_multiplier=0,
            )
            # e16r[j, ct*128+m] = (j == 2*ct + m//64): replicates rc rows
            nc.gpsimd.memset(e16r_s[:], 1.0)
            nc.gpsimd.affine_select(
                e16r_s[:], e16r_s[:], compare_op=ALU.is_equal, fill=0.0,
                base=0, pattern=[[2, NCT], [1, 2], [0, 64]],
                channel_multiplier=-1,
            )

            # zero padded edges of ktpad / vpad
            for ot in range(NQT):
                nc.gpsimd.memset(ktp_s[:, ot, 0:64], 0.0)
                nc.gpsimd.memset(ktp_s[:, ot, L + 64 : L + 128], 0.0)
            nc.gpsimd.memset(vp_s[0:64, 0, :], 0.0)
            nc.gpsimd.memset(vp_s[64:128, NKC - 1, :], 0.0)

            # ---- phases 1-3: projections + k-major attention ----
            # QK-projection for head pair hp+1 is interleaved into pair hp's
            # attention stream to keep TensorE dense (HAM stays at 2.4 GHz).
            with tc.tile_pool(name="psB", bufs=1, space="PSUM") as psB:
                psA = psB

                def emit_qk_proj(ot, lt):
                    ps = psA.tile([128, 512], F32, tag="proj", bufs=2,
                                  name=f"qkp{ot}_{lt}")
                    for ct in range(NCT):
                        nc.tensor.matmul(
                            ps[:],
                            lhsT=wqk_s[:, ct, ot * 128 : (ot + 1) * 128],
                            rhs=xT_s[:, ct, lt * 512 : (lt + 1) * 512],
                            start=(ct == 0), stop=(ct == NCT - 1),
                        )
                    if ot < NQT:
                        dest = qT_s[:, ot, lt * 512 : (lt + 1) * 512]
                    else:
                        dest = ktp_s[:, ot - NQT,
                                     64 + lt * 512 : 64 + (lt + 1) * 512]
                    nc.scalar.activation(
                        dest, ps[:], AF.Identity, bias=bqk_s[:, ot : ot + 1]
                    )

                # replicate v-bias across partitions: ones[128,1] @ bv[1,512]
                for nt in range(2):
                    ps = psA.tile([128, 512], F32, tag="proj", bufs=2,
                                  name=f"bvp{nt}")
                    nc.tensor.matmul(
                        ps[:], lhsT=ones1_s[:],
                        rhs=bv_row[:, nt * 512 : (nt + 1) * 512],
                        start=True, stop=True,
                    )
                    nc.scalar.copy(bvrep_s[:, nt * 512 : (nt + 1) * 512], ps[:])

                # Q/K projection for head pair 0 only; rest interleaved below
                for ot in (0, NQT):
                    for lt in range(2):
                        emit_qk_proj(ot, lt)

                # V projection -> vpad (seq-major, shifted +64)
                for lt in range(NQT):
                    for nt in range(2):
                        ps = psA.tile([128, 512], F32, tag="proj", bufs=2,
                                      name=f"vp{lt}_{nt}")
                        for ct in range(NCT):
                            nc.tensor.matmul(
                                ps[:],
                                lhsT=xT_s[:, ct, lt * 128 : (lt + 1) * 128],
                                rhs=wqk_s[:, ct,
                                          2 * C + nt * 512 : 2 * C + (nt + 1) * 512],
                                start=(ct == 0), stop=(ct == NCT - 1),
                            )
                        vtmp = work.tile([128, 512], BF16, tag="vtmp")
                        nc.vector.scalar_tensor_tensor(
                            out=vtmp[:], in0=ps[:], scalar=1.0,
                            in1=bvrep_s[:, nt * 512 : (nt + 1) * 512],
                            op0=ALU.mult, op1=ALU.add,
                        )
                        sl = slice(nt * 512, (nt + 1) * 512)
                        nc.sync.dma_start(vp_s[64:128, lt, sl], vtmp[0:64, :])
                        nc.sync.dma_start(vp_s[0:64, lt + 1, sl], vtmp[64:128, :])
                # rowsums for all heads in ONE psum bank: rows h (q<512)
                # and rows 32+h (q>=512)
                rs_ps = psB.tile([64, 512], F32, tag="rsall")
                nc.tensor.matmul(
                    rs_ps[:], lhsT=zrow_s[:, 0:64], rhs=zrow_s[:],
                    start=True, stop=True, skip_group_check=True,
                )

                def kc_geom(kc):
                    # query window cols for key chunk kc, in real q coords
                    q0 = max(kc * 128 - 128, 0)
                    q1 = min(kc * 128 + 128, L)
                    mi = 1 if kc == 0 else (2 if kc == NKC - 1 else 0)
                    return q0, q1, mi

                pend = []  # software pipeline: deferred AV/rs stages
                LAG = 2

                def do_av(st):
                    (h, hb, hp, kc, pm, q0, q1, ot_ps, first) = st
                    if first:  # zero-init this pair's ot banks
                        for bk in range(2):
                            nc.tensor.matmul(
                                ot_ps[:, bk * 512 : (bk + 1) * 512],
                                lhsT=zrow_s[:, 0:128], rhs=zrow_s[:],
                                start=True, stop=True, skip_group_check=True,
                            )
                    # split AV/rs matmuls at psum bank boundaries
                    spans = []
                    for b0 in (0, 512):
                        s0, s1 = max(q0, b0), min(q1, b0 + 512)
                        if s0 < s1:
                            spans.append((s0, s1))
                    for s0, s1 in spans:
                        j0, j1 = s0 - q0, s1 - q0
                        nc.tensor.matmul(
                            ot_ps[hb : hb + 64, s0:s1],
                            lhsT=vp_s[:, kc, h * 64 : (h + 1) * 64],
                            rhs=pm[:, j0:j1],
                            start=False, stop=(kc == NKC - 1),
                            skip_group_check=True,
                        )
                    for s0, s1 in spans:
                        j0, j1 = s0 - q0, s1 - q0
                        rb = 0 if s0 < 512 else 32
                        nc.tensor.matmul(
                            rs_ps[rb : rb + H, s0 - (512 if rb else 0)
                                  : s1 - (512 if rb else 0)],
                            lhsT=e16c_s[:, h, :],
                            rhs=pm[:, j0:j1],
                            start=False,
                            stop=(h == H - 1 and kc == NKC - 1),
                            skip_group_check=True,
                        )

                for hp in range(H // 2):
                    # queue next pair's QK-projection, interleaved below
                    projq = []
                    if hp + 1 < H // 2:
                        for ot in (hp + 1, NQT + hp + 1):
                            for lt in range(2):
                                projq.append((ot, lt))
                    ot_ps = psB.tile([128, L], F32, tag="ot", bufs=1,
                                     name=f"ot_ps{hp}")
                    for hh in range(2):
                        hb = hh * 64
                        h = hp * 2 + hh
                        for kc in range(NKC):
                            if projq and kc % 4 == 1:
                                emit_qk_proj(*projq.pop(0))
                            q0, q1, mi = kc_geom(kc)
                            w = q1 - q0
                            s_ps = psB.tile([128, 256], F32, tag="s", bufs=3,
                                            name=f"s{h}_{kc}")
                            nc.tensor.matmul(
                                s_ps[:, 0:w],
                                lhsT=ktp_s[hb : hb + 64, hp,
                                           kc * 128 : (kc + 1) * 128],
                                rhs=qT_s[hb : hb + 64, hp, q0:q1],
                                start=True, stop=True,
                            )
                            p_sb = work.tile([128, 256], BF16, tag="p")
                            nc.scalar.activation(p_sb[:, 0:w], s_ps[:, 0:w],
                                                 AF.Exp, scale=SCALE)
                            pm = work.tile([128, 256], BF16, tag="pm", bufs=4)
                            nc.vector.tensor_mul(pm[:, 0:w], p_sb[:, 0:w],
                                                 masks_s[:, mi, 0:w])
                            pend.append((h, hb, hp, kc, pm, q0, q1, ot_ps,
                                         hh == 0 and kc == 0))
                            if len(pend) > LAG:
                                do_av(pend.pop(0))
                    while pend:
                        do_av(pend.pop(0))
                    # evacuate this pair's O^T (unnormalized)
                    for lt in range(2):
                        nc.vector.tensor_copy(
                            oT_s[:, hp, lt * 512 : (lt + 1) * 512],
                            ot_ps[:, lt * 512 : (lt + 1) * 512],
                        )

                # deferred softmax normalization
                nc.vector.reciprocal(rc_f32[:, 0:512], rs_ps[0:H, :])
                nc.vector.reciprocal(rc_f32[:, 512:L], rs_ps[32 : 32 + H, :])
                nc.vector.tensor_copy(rc_bf[:], rc_f32[:])

            # ---- phase 4: normalize oT + out projection -> y ----
            with tc.tile_pool(name="psC", bufs=2, space="PSUM") as psC:
                for ct in range(NCT):
                    for lt in range(2):
                        rep = psC.tile([128, 512], F32, tag="rep")
                        nc.tensor.matmul(
                            rep[:],
                            lhsT=e16r_s[:, ct * 128 : (ct + 1) * 128],
                            rhs=rc_bf[:, lt * 512 : (lt + 1) * 512],
                            start=True, stop=True,
                        )
                        nc.vector.tensor_mul(
                            oT_s[:, ct, lt * 512 : (lt + 1) * 512], rep[:],
                            oT_s[:, ct, lt * 512 : (lt + 1) * 512],
                        )
                for lt in range(NQT):
                    for mt in range(2):
                        ps = psC.tile([128, 512], F32, tag="proj")
                        for ct in range(NCT):
                            nc.tensor.matmul(
                                ps[:],
                                lhsT=oT_s[:, ct, lt * 128 : (lt + 1) * 128],
                                rhs=wo_s[:, ct, mt * 512 : (mt + 1) * 512],
                                start=(ct == 0), stop=(ct == NCT - 1),
                            )
                        yb = work.tile([128, 512], F32, tag="yb")
                        nc.scalar.copy(yb[:], ps[:])
                        nc.sync.dma_start(
                            y_d[lt * 128 : (lt + 1) * 128,
                                mt * 512 : (mt + 1) * 512],
                            yb[:],
                        )

    nc.compile()
    return nc


def _get_nc():
    if "nc" not in _CACHED:
        _CACHED["nc"] = _build_nc()
    return _CACHED["nc"]


def _prep_in_maps(x, in_proj_w, in_proj_b, out_w):
    bf = ml_dtypes.bfloat16
    wqkT = np.ascontiguousarray(in_proj_w.T).astype(bf)
    woT = np.ascontiguousarray(out_w.T).astype(bf)
    bqk = np.ascontiguousarray(in_proj_b[: 2 * C]).astype(np.float32)
    bv = np.ascontiguousarray(in_proj_b[2 * C :]).astype(np.float32)
    in_maps = []
    for b in range(B):
        xT = np.ascontiguousarray(x[b].T).astype(bf)
        in_maps.append(
            {"xT": xT, "wqkT": wqkT, "woT": woT, "bqk": bqk, "bv": bv}
        )
    return in_maps


def kernel(x, in_proj_w, in_proj_b, out_w, out_b, _trace=False):
    nc = _get_nc()
    in_maps = _prep_in_maps(x, in_proj_w, in_proj_b, out_w)
    res = run_bass_kernel_spmd(nc, in_maps, list(range(NCORES)), trace=_trace)
    _CACHED["last_result"] = res
    y = np.stack([res.results[i]["y"] for i in range(NCORES)], axis=0)
    return (y + out_b[None, None, :].astype(np.float32)).astype(np.float32)
